# revision 1
# baseline (speedup 1.0000x reference)
"""Trainium2 Bass kernel for nn_AttentionHead (B=8, S=4096, D=128).

Sharding: data-parallel over the batch dim — 1 batch element per NeuronCore,
8 cores, SPMD (same NEFF, different x slice), weights replicated. No
collectives.

Per-core pipeline (S=4096 seq, D=128 head dim, all-on-chip, f16 compute
with f32 PSUM accumulation; fro rel err vs fp32 reference ~5e-4):
  1. x [4096,128] f32 -> cast-load f16 (SWDGE cast DMA) -> chunked
     DMA-xbar-transposes -> xT [d, s] f16 (chunked so projections start
     after the first quarter)
  2. q/k/v projections: matmul(lhsT=xT s-tile, rhs=W^T) -> PSUM f32.
     Two passes: (A) per tile stage raw q/k to SBUF f16 (ACT) with
     bn_stats/bn_aggr reading the staged f16 (DVE), v copies split
     ACT/DVE by parity (pass A is otherwise DVE-bound); then one batched
     rsqrt for all 64 rows via
     exp(-0.5*ln(var+eps)) — Ln/Exp share one ACT table set with the
     attention Exp, so the whole kernel needs ~3 table loads (a per-tile
     Sqrt thrashes 3.6us table reloads against the attention Exps);
     (B) apply LN on DVE (per-partition scale + broadcast bias) into
     [s,t,h] staging, one batched DMA-xbar-transpose per tensor to [h,t,s],
     then a single big ACT op folds LN weight/bias (per-partition scalars
     after the transpose). ALL xbar transposes go on the sync HWDGE ring:
     concurrent transposes on the sync+scalar rings corrupt data on HW.
  3. attention per 128-query i-tile:
     - scores = qT_tile^T @ kT (N=512 matmuls pairwise into 2-bank PSUM
       tiles; fp16 at 1 cycle/row, fp32 would be 4x slower)
     - exp via ACT directly off PSUM with scale=1/sqrt(D). No max
       subtraction: scores are ~N(0,1) (LN'd q,k), exp stays in f16 range.
     - DMA-xbar-transpose exp [i,j] -> expT [j,i] (f16, 1MB per i-tile)
     - PV: out[i,h] accumulates 32 matmuls lhsT=expT chunk, rhs=v chunk
       (v carries an appended ones column, so column 128 of the PSUM
       accumulator is the softmax row-sum -- the denominators fall out of
       the PV matmuls and the exp ops need no accum_out); normalize by
       1/rowsum (per-partition scalar) in the PSUM->SBUF copy on DVE.

All SBUF pools stay open for the whole kernel (no SBUF slot reuse across
phases): SBUF-space reuse attaches release waits to the DMAs that load into
recycled space, and walrus rejects DMAs with more than a couple of sync
waits ("Too many sync wait commands"). Only PSUM pools are scoped.
"""

import math

import numpy as np

from concourse import bacc
import concourse.mybir as mybir
import concourse.tile as tile
from concourse.bass_utils import run_bass_kernel_spmd

F16 = mybir.dt.float16
F32 = mybir.dt.float32
AF = mybir.ActivationFunctionType
ALU = mybir.AluOpType

B, S, D = 8, 4096, 128
P = 128
NT = S // P  # 32 s-tiles
EPS = 1e-5
ISQRT_D = 1.0 / math.sqrt(D)
N_CORES = 8
_ABLATE = set()  # timing-ablation flags, empty in production
JC = 1024  # key-chunk width for the exp pass (2 PSUM banks)
NJC = S // JC


def _ln_param_to_sbuf(nc, pool, dram_ap, tag):
    t = pool.tile([P, 1], F32, tag=tag)
    nc.sync.dma_start(t, dram_ap[:, None])
    return t


def _build_attention(tc, out_d, x_d, w_d, ln_d):
    """Emit the single-core attention program.

    out_d: [S, D] f32 output AP.  x_d: [S, D] f32 input AP.
    w_d: dict q/k/v -> [D, D] f32 weight AP (torch Linear layout: out = x @ W^T).
    ln_d: dict qw/qb/kw/kb -> [D] f32 LN param APs.
    """
    nc = tc.nc

    with (
        tc.tile_pool(name="const", bufs=1) as const,
        tc.tile_pool(name="big", bufs=1) as big,
        tc.tile_pool(name="wtmp", bufs=3) as wtmp,
        tc.tile_pool(name="xload", bufs=1) as xload,
        tc.tile_pool(name="stat", bufs=6) as stat,
        tc.tile_pool(name="attn", bufs=3) as attn,
        tc.tile_pool(name="small", bufs=4) as small,
    ):
        # --- weights: load [h,d] f32, cast f16, DMA-xbar-transpose -> W^T f16
        WT = {}
        for name in ("q", "k", "v"):
            w32 = wtmp.tile([P, P], F32, tag=f"w32_{name}")
            nc.sync.dma_start(w32, w_d[name])
            w16 = wtmp.tile([P, P], F16, tag=f"w16_{name}")
            nc.vector.tensor_copy(w16, w32)
            wt = const.tile([P, P], F16, tag=f"wt_{name}")
            nc.sync.dma_start_transpose(wt, w16)
            WT[name] = wt

        # --- x load (cast f32->f16 in the SWDGE DMA), one batched
        # DMA-xbar-transpose: xT[d, t, s] = x16[s, t*128+d]
        xT = big.tile([P, NT, P], F16, tag="xT")  # [d, t, s%128]
        x16 = xload.tile([P, NT, P], F16)  # [s%128, t, d]
        _ab = _ABLATE
        # load and transpose in quarters so the first projections start
        # as soon as the first 8 s-tiles are resident
        x_r = x_d.rearrange("(t p) d -> p t d", p=P)
        x16f = x16.rearrange("p t d -> p (t d)")
        for c in range(4):
            nc.gpsimd.dma_start(x16[:, c * 8:(c + 1) * 8, :],
                                x_r[:, c * 8:(c + 1) * 8, :])
            nc.sync.dma_start_transpose(
                xT[:, c * 8:(c + 1) * 8, :],
                x16f[:, c * 8 * P:(c + 1) * 8 * P])

        # LN params loaded after the x/weight ring traffic: they head-of-line
        # block the sync ring for ~2us if issued first, and aren't needed
        # until pass B
        qnw = _ln_param_to_sbuf(nc, const, ln_d["qw"], "qnw")
        qnb = _ln_param_to_sbuf(nc, const, ln_d["qb"], "qnb")
        knw = _ln_param_to_sbuf(nc, const, ln_d["kw"], "knw")
        knb = _ln_param_to_sbuf(nc, const, ln_d["kb"], "knb")

        # --- projections + layernorm -> qT, kT [h, s] f16; v [s, h] f16
        # Two passes: (A) project q/k/v, stage raw q/k + bn stats; then ONE
        # batched rsqrt for all 64 (tile, tensor) rows via exp(-0.5*ln(v+eps))
        # (a single Ln + Exp keeps ACT table switching to ~2 loads; per-tile
        # Sqrt thrashes table sets against the attention Exp ops); (B) apply
        # LN, transpose, fold ln weight/bias.
        qT = big.tile([P, NT, P], F16, tag="qT")
        kT = big.tile([P, NT, P], F16, tag="kT")
        v16 = big.tile([P, NT, P + 1], F16, tag="v16")  # [:, t, P] = 1.0
        nc.vector.memset(v16[:, :, P:P + 1], 1.0)
        raw = big.tile([P, 2 * NT, P], F16, tag="raw")  # pre-LN q/k
        mvall = big.tile([P, 2 * NT, 2], F32, tag="mvall")  # (mean, var)
        rstd_all = big.tile([P, 2 * NT], F32, tag="rstd_all")
        nmr_all = big.tile([P, 2 * NT], F32, tag="nmr_all")
        with tc.tile_pool(name="pps", bufs=2, space="PSUM") as pps:
            for t in range(NT if "ph12" not in _ab else 0):
                for k, name in enumerate(("q", "k", "v")):
                    ps = pps.tile([P, P], F32, tag=f"p_{name}")
                    nc.tensor.matmul(ps, lhsT=xT[:, t, :], rhs=WT[name],
                                     start=True, stop=True)
                    if name == "v":
                        # split the PSUM evacuations across ACT/DVE: pass A
                        # is otherwise DVE-bound while ACT has headroom
                        if t % 2 == 0:
                            nc.vector.tensor_copy(v16[:, t, :P], ps)
                        else:
                            nc.scalar.activation(v16[:, t, :P], ps, AF.Copy)
                    else:
                        idx = 2 * t + k
                        nc.scalar.activation(raw[:, idx, :], ps, AF.Copy)
                        # stats from the staged f16 raw (SBUF read is cheaper
                        # on DVE than PSUM, and the stats then describe
                        # exactly the values pass B normalizes)
                        st = stat.tile([P, 6], F32, tag="st")
                        nc.vector.bn_stats(st, raw[:, idx, :])
                        nc.vector.bn_aggr(mvall[:, idx, :], st)
        if "ph12" not in _ab:
            vare = stat.tile([P, 2 * NT], F32, tag="vare")
            nc.vector.tensor_scalar_add(vare, mvall[:, :, 1], EPS)
            # rsqrt(v) = exp(-0.5 * ln(v)), batched over all 64 rows
            nc.scalar.activation(rstd_all, vare, AF.Ln)
            nc.scalar.activation(rstd_all, rstd_all, AF.Exp, scale=-0.5)
            # (-mean) * rstd
            nc.vector.scalar_tensor_tensor(
                nmr_all, in0=mvall[:, :, 0], scalar=-1.0, in1=rstd_all,
                op0=ALU.mult, op1=ALU.mult)
        s1q = big.tile([P, NT, P], F16, tag="s1q")
        s1k = big.tile([P, NT, P], F16, tag="s1k")
        qT_pre = big.tile([P, NT, P], F16, tag="qT_pre")
        kT_pre = big.tile([P, NT, P], F16, tag="kT_pre")
        for s1all, koff in ((s1k, 1), (s1q, 0)):
            for t in range(NT if "ph12" not in _ab else 0):
                idx = 2 * t + koff
                nc.vector.scalar_tensor_tensor(
                    s1all[:, t, :], in0=raw[:, idx, :],
                    scalar=rstd_all[:, idx:idx + 1],
                    in1=nmr_all[:, idx:idx + 1].to_broadcast([P, P]),
                    op0=ALU.mult, op1=ALU.add)
        if "ph12" not in _ab:
            # k first: attention needs ALL of kT but only early qT tiles.
            # wb is applied per transpose-half so it pipelines with the
            # (serialized) sync-ring transposes.
            for s1all, pre, Tdst, wsb, bsb in (
                (s1k, kT_pre, kT, knw, knb),
                (s1q, qT_pre, qT, qnw, qnb),
            ):
                s1f = s1all.rearrange("p t h -> p (t h)")
                T2 = Tdst.rearrange("h t s -> h (t s)")
                P2 = pre.rearrange("h t s -> h (t s)")
                for hh in range(2):
                    nc.sync.dma_start_transpose(
                        pre[:, hh * 16:(hh + 1) * 16, :],
                        s1f[:, hh * 16 * P:(hh + 1) * 16 * P])
                    nc.scalar.activation(
                        T2[:, hh * 16 * P:(hh + 1) * 16 * P],
                        P2[:, hh * 16 * P:(hh + 1) * 16 * P],
                        AF.Identity, scale=wsb, bias=bsb)

        # --- attention
        kT2 = kT.rearrange("h t s -> h (t s)")
        with (
            tc.tile_pool(name="scps", bufs=3, space="PSUM") as scps,
            tc.tile_pool(name="pvps", bufs=2, space="PSUM") as pvps,
        ):
            # Software-pipelined: PV for tile i is emitted AFTER tile i+1's
            # QK/exp, so the PE runs QK[i+1] (which feeds the ACT-critical
            # exp stream) before PV[i] — otherwise ACT stalls ~1.2us/tile
            # waiting on QK behind PV in the PE FIFO.
            def emit_qk_exp(i):
                exp_nat = attn.tile([P, S], F16, tag="expn")  # [i, j]
                expT = attn.tile([P, NT, P], F16, tag="expt")  # [j%128, jt, i]
                for jc in range(NJC):
                    sc = scps.tile([P, JC], F32, tag="sc")
                    if "qk" not in _ab:
                        for h in range(JC // 512):
                            nc.tensor.matmul(
                                sc[:, h * 512:(h + 1) * 512], lhsT=qT[:, i, :],
                                rhs=kT2[:, jc * JC + h * 512:jc * JC + (h + 1) * 512],
                                start=True, stop=True)
                    if "exp" not in _ab:
                        nc.scalar.activation(
                            exp_nat[:, jc * JC:(jc + 1) * JC], sc, AF.Exp,
                            scale=ISQRT_D)
                    if "tdma" not in _ab and jc % 2 == 1:
                        # transpose each half as soon as its exps land (sync
                        # ring only: concurrent xbar transposes on the two
                        # HWDGE rings corrupt data on HW, bisected)
                        hh = jc // 2
                        nc.sync.dma_start_transpose(
                            expT[:, hh * 16:(hh + 1) * 16, :],
                            exp_nat[:, hh * 2 * JC:(hh + 1) * 2 * JC])
                return exp_nat, expT

            def emit_pv(i, exp_nat, expT):
                # PV with a ones column appended to v: ops[:, P] accumulates
                # sum_j exp[i, j], so the softmax denominator falls out of
                # the same matmuls (no ACT accum_out needed: -187ns/exp op)
                ops = pvps.tile([P, P + 1], F32, tag="pv")
                if "pv" not in _ab:
                    lsrc = (expT if "tdma" not in _ab
                            else exp_nat.rearrange("p (t s) -> p t s", s=P))
                    for c in range(NT):
                        nc.tensor.matmul(ops, lhsT=lsrc[:, c, :],
                                         rhs=v16[:, c, :],
                                         start=(c == 0), stop=(c == NT - 1))
                rsum = small.tile([P, 1], F32, tag="rsum")
                nc.vector.reciprocal(rsum, ops[:, P:P + 1])
                osb = small.tile([P, P], F32, tag="osb")
                nc.vector.tensor_scalar_mul(osb, ops[:, :P], rsum)
                nc.gpsimd.dma_start(out_d[i * P:(i + 1) * P, :], osb)

            prev = None
            for i in range(NT):
                cur = emit_qk_exp(i)
                if prev is not None:
                    emit_pv(i - 1, *prev)
                prev = cur
            emit_pv(NT - 1, *prev)


_NC_CACHE = None


def _build():
    global _NC_CACHE
    if _NC_CACHE is not None:
        return _NC_CACHE
    nc = bacc.Bacc("TRN2", target_bir_lowering=False, debug=False)
    x = nc.dram_tensor("x", [S, D], F32, kind="ExternalInput").ap()
    wq = nc.dram_tensor("Wq", [D, D], F32, kind="ExternalInput").ap()
    wk = nc.dram_tensor("Wk", [D, D], F32, kind="ExternalInput").ap()
    wv = nc.dram_tensor("Wv", [D, D], F32, kind="ExternalInput").ap()
    qn_w = nc.dram_tensor("qn_w", [D], F32, kind="ExternalInput").ap()
    qn_b = nc.dram_tensor("qn_b", [D], F32, kind="ExternalInput").ap()
    kn_w = nc.dram_tensor("kn_w", [D], F32, kind="ExternalInput").ap()
    kn_b = nc.dram_tensor("kn_b", [D], F32, kind="ExternalInput").ap()
    out = nc.dram_tensor("out", [S, D], F32, kind="ExternalOutput").ap()
    with tile.TileContext(nc) as tc:
        _build_attention(
            tc, out, x,
            {"q": wq, "k": wk, "v": wv},
            {"qw": qn_w, "qb": qn_b, "kw": kn_w, "kb": kn_b},
        )
    nc.compile()
    _NC_CACHE = nc
    return nc


def kernel(x, Wq, Wk, Wv, qn_w, qn_b, kn_w, kn_b, _run_kwargs=None):
    nc = _build()
    x = np.asarray(x, dtype=np.float32)
    shared = {
        "Wq": np.ascontiguousarray(np.asarray(Wq, np.float32)),
        "Wk": np.ascontiguousarray(np.asarray(Wk, np.float32)),
        "Wv": np.ascontiguousarray(np.asarray(Wv, np.float32)),
        "qn_w": np.ascontiguousarray(np.asarray(qn_w, np.float32)),
        "qn_b": np.ascontiguousarray(np.asarray(qn_b, np.float32)),
        "kn_w": np.ascontiguousarray(np.asarray(kn_w, np.float32)),
        "kn_b": np.ascontiguousarray(np.asarray(kn_b, np.float32)),
    }
    in_maps = [
        {"x": np.ascontiguousarray(x[b]), **shared} for b in range(B)
    ]
    res = run_bass_kernel_spmd(nc, in_maps, core_ids=list(range(N_CORES)),
                               **(_run_kwargs or {}))
    out = np.stack([res.results[b]["out"] for b in range(B)], axis=0)
    if _run_kwargs:
        kernel.last_results = res
    return out.astype(np.float32)

